# revision 21
# baseline (speedup 1.0000x reference)
"""Multi-head attention (B=2, S=2048, E=1024, H=16, causal) on 8 Trainium2 cores.

Sharding: data-parallel over batch (2) x tensor-parallel over heads (4 groups
of 4 heads). Core i handles batch i//4, heads 4*(i%4) .. 4*(i%4)+3.
Each core computes Q/K/V projections for its 256 channels, causal
flash-attention for its 4 heads, and a partial output projection
(contribution of its channels to all 1024 output features). Partials are
summed across the 4 cores of each batch group (host-side).

Performance structure (see HAM clock gate: PE idles -> 1.2 GHz, sustained
busy -> 2.4 GHz after ~3.4us):
 - inputs are host-prefolded into SBUF partition layout so the whole input
   stream is ~10 large contiguous-row DMA descriptors (descriptor issue on
   the sync engine costs ~0.6us each and paced the old kernel's whole
   projection phase);
 - dummy matmuls on memset data warm the PE during the ~10us DMA startup so
   real work runs at full clock from the first instruction;
 - all projections (Q/K both d-tiles, V all 16 s-tiles) run dense in a
   single phase on resident data; attention keeps the PE packed with
   out-projection fillers while ACT paces the exp chain;
 - the last q-block's norm + out-projection + store is pipelined per s-tile
   to shrink the serial tail.
"""
import numpy as np

import concourse.bass as bass
import concourse.tile as tile
from concourse import bacc, mybir
from concourse.bass_utils import run_bass_kernel_spmd

F32 = mybir.dt.float32
F32R = mybir.dt.float32r
BF16 = mybir.dt.bfloat16
import ml_dtypes
MM_NP = ml_dtypes.bfloat16
ActF = mybir.ActivationFunctionType
Alu = mybir.AluOpType

B, S, E = 2, 2048, 1024
H, DH = 16, 64
NCORES, TPW = 8, 4          # 8 cores, 4-way tensor parallel per batch
HPC = H // TPW              # heads per core = 4
C = HPC * DH                # channels per core = 256
SCALE = 1.0 / 8.0           # 1/sqrt(DH)
VW = HPC * (DH + 1)         # V storage width per s-tile (ones col per head)
NST = S // 128              # 16 s-tiles of 128 rows
NQB = S // 512              # 4 q-blocks of 512
NEC = E // 128              # 8 e-chunks (contraction for projections)
WQKV = 3 * C                # 768 cols of folded W per e-chunk
NDUMMY = 40                 # PE warmup matmuls during DMA startup

_cache = {}


def _emit(nc, tc, causal):
    # ---- DRAM parameters (host-prefolded to SBUF partition layout) ----
    # xt: col = sb*4096 + ec*512 + sl  ->  X^T[ec*128+p, sb*512+sl]
    xt_d = nc.dram_tensor("xt", [128, NQB * NEC * 512], BF16,
                          kind="ExternalInput").ap()
    # wqkv: col = ec*768 + {0:wq,256:wk,512:wv}*256 + c
    wqkv_d = nc.dram_tensor("wqkv", [128, NEC * WQKV], BF16,
                            kind="ExternalInput").ap()
    # wot: col = cc*1024 + e
    wot_d = nc.dram_tensor("wot", [128, 2 * E], BF16, kind="ExternalInput").ap()
    bqk_d = nc.dram_tensor("bqk", [128, 4], F32, kind="ExternalInput").ap()
    bv_d = nc.dram_tensor("bv", [1, C], F32, kind="ExternalInput").ap()
    bo_d = nc.dram_tensor("bo", [1, E], F32, kind="ExternalInput").ap()
    ones_d = nc.dram_tensor("ones", [1, 128], F32, kind="ExternalInput").ap()
    onesv_d = nc.dram_tensor("onesv", [128, NST * HPC], F32,
                             kind="ExternalInput").ap()
    out_d = nc.dram_tensor("out", [S, E], BF16, kind="ExternalOutput").ap()

    ctxpool = tc.tile_pool

    with ctxpool(name="persist", bufs=1) as pp:
        # ---- persistent SBUF tensors ----
        xt_sb = pp.tile([128, NQB * NEC * 512], BF16)
        wqkv_sb = pp.tile([128, NEC * WQKV], BF16)
        wot_sb = pp.tile([128, 2 * E], BF16)         # c-chunk cc at cols [cc*E)
        qt_sb = pp.tile([128, 2 * S], BF16)          # Q^T, d-tile t at cols [t*S)
        kt_sb = pp.tile([128, 2 * S], BF16)
        v_sb = pp.tile([128, NST * VW], BF16)        # V (+ones col per head)
        ot_sb = pp.tile([128, 2 * S], BF16)          # normalized attn out^T
        bqk_sb = pp.tile([128, 4], F32)
        bvb_sb = pp.tile([128, C], F32)              # bv broadcast to partitions
        bob_sb = pp.tile([128, E], F32)              # bo broadcast to partitions
        ones_r = pp.tile([1, 128], F32R)
        dmy_sb = pp.tile([128, 512], BF16)           # PE warmup fodder

        def xt_cols(sb, ec, off, width):
            c0 = sb * (NEC * 512) + ec * 512 + off
            return xt_sb[:, c0: c0 + width]

        def wv_cols(ec):
            return wqkv_sb[:, ec * WQKV + 2 * C: ec * WQKV + 3 * C]

        def emit_vproj(psum_pool, st, vtag="mps"):
            """Project V for s-tile st into v_sb (with per-head ones column)."""
            sb, off = st // 4, (st % 4) * 128
            ps = psum_pool.tile([128, C], F32, tag=vtag, name=f"vp{st}")
            for ec in range(NEC):
                nc.tensor.matmul(
                    ps[:], xt_cols(sb, ec, off, 128), wv_cols(ec),
                    start=(ec == 0), stop=(ec == NEC - 1),
                    skip_group_check=True)
            dst = v_sb[:, st * VW: st * VW + VW].rearrange(
                "p (h x) -> p h x", h=HPC)[:, :, 0:DH]
            nc.vector.tensor_add(
                dst,
                ps[:].rearrange("p (h x) -> p h x", h=HPC),
                bvb_sb[:].rearrange("p (h x) -> p h x", h=HPC))

        with ctxpool(name="small", bufs=1) as sp:
            bv_row = sp.tile([1, C], F32R)
            bo_row = sp.tile([1, E], F32R)
            onesb_sb = sp.tile([128, NST * HPC], F32)

            # ---- input DMAs: few big descriptors, consumption order.
            # sync queue: weights + X stream; small tensors go on the
            # vector/gpsimd queues so their issue overlaps. ----
            nc.sync.dma_start(out=wqkv_sb[:, 0: WQKV],
                              in_=wqkv_d[:, 0: WQKV])
            nc.sync.dma_start(out=xt_sb[:, 0: 2 * 512],
                              in_=xt_d[:, 0: 2 * 512])
            nc.sync.dma_start(out=wqkv_sb[:, WQKV: 4 * WQKV],
                              in_=wqkv_d[:, WQKV: 4 * WQKV])
            nc.sync.dma_start(out=xt_sb[:, 2 * 512: NEC * 512],
                              in_=xt_d[:, 2 * 512: NEC * 512])
            nc.sync.dma_start(out=wqkv_sb[:, 4 * WQKV: NEC * WQKV],
                              in_=wqkv_d[:, 4 * WQKV: NEC * WQKV])
            for sb in range(1, NQB):
                nc.sync.dma_start(
                    out=xt_sb[:, sb * NEC * 512: (sb + 1) * NEC * 512],
                    in_=xt_d[:, sb * NEC * 512: (sb + 1) * NEC * 512])
            nc.sync.dma_start(out=wot_sb[:], in_=wot_d[:])
            nc.scalar.dma_start(out=bqk_sb[:], in_=bqk_d[:])
            nc.scalar.dma_start(out=ones_r[:], in_=ones_d[:].bitcast(F32R))
            nc.scalar.dma_start(out=onesb_sb[:], in_=onesv_d[:])
            nc.gpsimd.dma_start(out=bv_row[:], in_=bv_d[:].bitcast(F32R))
            nc.gpsimd.dma_start(out=bo_row[:], in_=bo_d[:].bitcast(F32R))

            # V ones columns via a strided DVE copy (a strided DMA here costs
            # ~10us of descriptor generation and blocks the ring)
            v_ones_ap = v_sb[:].rearrange("p (n x) -> p n x", x=DH + 1)[:, :, DH:DH + 1]
            nc.vector.tensor_copy(
                v_ones_ap, onesb_sb[:].rearrange("p (n x) -> p n x", x=1))

            # ==== phase A: PE warmup on memset data (keeps the HAM clock
            # gate open through the DMA startup; no DMA dependencies) ====
            with ctxpool(name="warm_ps", bufs=1, space="PSUM") as warm_ps:
                with tc.high_priority(offset=-3_000_000):
                    nc.vector.memset(dmy_sb[:], 0.0)
                    wps = warm_ps.tile([128, 512], F32, tag="wm")
                    for i in range(6):
                        nc.tensor.matmul(wps[:], dmy_sb[:, 0:128], dmy_sb[:],
                                         start=True, stop=True,
                                         skip_group_check=True)
                    for i in range(10):
                        nc.tensor.matmul(wps[:, 0:128], dmy_sb[:, 0:128],
                                         dmy_sb[:, 0:128],
                                         start=True, stop=True,
                                         skip_group_check=True)

            # ==== phase B: sb0 projections only; sb1-3 drip into the
            # attention phase as fillers so the exp chain overlaps them ====
            sb_inline = 1 if causal else NQB
            with ctxpool(name="proj_ps", bufs=4, space="PSUM") as proj_ps, \
                 ctxpool(name="vp_ps", bufs=2, space="PSUM") as vp_ps:
                first_group = True
                for sb in range(sb_inline):
                    for pj in range(2):       # 0 = Q, 1 = K
                        for dt in range(2):   # d-tile (head pair)
                            psq = proj_ps.tile([128, 512], F32, tag="pps",
                                               name=f"pp{sb}{pj}{dt}")
                            for ec in range(NEC):
                                w0 = ec * WQKV + pj * C + dt * 128
                                nc.tensor.matmul(
                                    psq[:], wqkv_sb[:, w0: w0 + 128],
                                    xt_cols(sb, ec, 0, 512),
                                    start=(ec == 0), stop=(ec == NEC - 1),
                                    skip_group_check=True)
                            o_sb = qt_sb if pj == 0 else kt_sb
                            nc.vector.tensor_scalar_add(
                                o_sb[:, dt * S + sb * 512: dt * S + sb * 512 + 512],
                                psq[:], bqk_sb[:, 2 * pj + dt: 2 * pj + dt + 1])
                            if first_group:
                                # bias broadcasts (ones/bv/bo land ~10us via
                                # the vector/gpsimd queues)
                                first_group = False
                                ps_bv = vp_ps.tile([128, C], F32, tag="vps")
                                nc.tensor.matmul(ps_bv[:], ones_r[0:1, 0:128],
                                                 bv_row[:], start=True, stop=True)
                                nc.vector.tensor_copy(bvb_sb[:], ps_bv[:])
                                for eb in range(2):
                                    ps_bo = vp_ps.tile([128, 512], F32, tag="vps",
                                                       name=f"bo{eb}")
                                    nc.tensor.matmul(
                                        ps_bo[:], ones_r[0:1, 0:128],
                                        bo_row[0:1, eb * 512:(eb + 1) * 512],
                                        start=True, stop=True)
                                    nc.vector.tensor_copy(
                                        bob_sb[:, eb * 512:(eb + 1) * 512],
                                        ps_bo[:])
                    for st in range(4 * sb, 4 * sb + 4):
                        emit_vproj(vp_ps, st, vtag="vps")

            # ==== phase C: attention (q-block outer, head inner) + out-proj ====
            with ctxpool(name="score_ps", bufs=2, space="PSUM") as score_ps, \
                 ctxpool(name="attn_ps", bufs=2, space="PSUM") as attn_ps, \
                 ctxpool(name="misc_ps", bufs=2, space="PSUM") as misc_ps, \
                 ctxpool(name="pt_pool", bufs=10) as pt_pool, \
                 ctxpool(name="rec_pool", bufs=4) as rec_pool, \
                 ctxpool(name="bc_pool", bufs=4) as bc_pool, \
                 ctxpool(name="out_pool", bufs=8) as out_pool:
                ot_half = {}
                pending = []   # deferred norm closures of the previous hp
                # filler queue: (deadline_qb, closure). Closures emit ~1
                # matmul each and drip one-per-attention-step so the PE
                # always has work while ACT paces the exp chain.
                filler_q = []
                vp_tiles = {}

                def flush_pending():
                    while pending:
                        pending.pop(0)()

                warm_ctr = [0]

                def drip(n=1):
                    for _ in range(n):
                        if not filler_q:
                            # keep the HAM clock gate open: a tiny matmul on
                            # warm fodder instead of letting the PE idle
                            warm_ctr[0] += 1
                            if warm_ctr[0] % 2 == 0:
                                with tc.high_priority(offset=-1_000_000):
                                    ps_w = misc_ps.tile(
                                        [128, 128], F32, tag="mps",
                                        name=f"wrm{warm_ctr[0]}")
                                    nc.tensor.matmul(
                                        ps_w[:], dmy_sb[:, 0:128],
                                        dmy_sb[:, 0:128],
                                        start=True, stop=True,
                                        skip_group_check=True)
                            return
                        _, fn = filler_q.pop(0)
                        with tc.high_priority(offset=-1_000_000):
                            fn()

                def drain_due(qb):
                    while filler_q and filler_q[0][0] <= qb:
                        _, fn = filler_q.pop(0)
                        with tc.high_priority(offset=-1_000_000):
                            fn()

                def queue_vproj(st):
                    # two closures: open the 8-step accumulation, then
                    # finish it + bias-add into v_sb
                    sb, off = st // 4, (st % 4) * 128
                    def part_a(st=st, sb=sb, off=off):
                        ps = misc_ps.tile([128, C], F32, tag="mps",
                                          name=f"vp{st}")
                        vp_tiles[st] = ps
                        for ec in range(4):
                            nc.tensor.matmul(
                                ps[:], xt_cols(sb, ec, off, 128), wv_cols(ec),
                                start=(ec == 0), stop=False,
                                skip_group_check=True)
                    def part_b(st=st, sb=sb, off=off):
                        ps = vp_tiles.pop(st)
                        for ec in range(4, NEC):
                            nc.tensor.matmul(
                                ps[:], xt_cols(sb, ec, off, 128), wv_cols(ec),
                                start=False, stop=(ec == NEC - 1),
                                skip_group_check=True)
                        dst = v_sb[:, st * VW: st * VW + VW].rearrange(
                            "p (h x) -> p h x", h=HPC)[:, :, 0:DH]
                        nc.vector.tensor_add(
                            dst, ps[:].rearrange("p (h x) -> p h x", h=HPC),
                            bvb_sb[:].rearrange("p (h x) -> p h x", h=HPC))
                    dl = st // 4
                    filler_q.append((dl, part_a))
                    filler_q.append((dl, part_b))

                def queue_outproj_cc0(qb):
                    # first-half out-projection (heads 0,1 -> cc 0)
                    for st in range(qb * 4, qb * 4 + 4):
                        def fn(st=st):
                            o_t = out_pool.tile([128, E], BF16, tag="ob",
                                                name=f"ot{st}")
                            ot_half[st] = o_t
                            for eb in range(2):
                                ps_f = misc_ps.tile([128, 512], F32, tag="mps",
                                                    name=f"pg{st}{eb}")
                                nc.tensor.matmul(
                                    ps_f[:],
                                    ot_sb[:, st * 128: st * 128 + 128],
                                    wot_sb[:, eb * 512: eb * 512 + 512],
                                    start=True, stop=True)
                                nc.vector.tensor_add(
                                    o_t[:, eb * 512:(eb + 1) * 512], ps_f[:],
                                    bob_sb[:, eb * 512:(eb + 1) * 512])
                        filler_q.append((NQB, fn))

                def queue_outproj_cc1(qb):
                    # second half (heads 2,3 -> cc 1) + store
                    for st in range(qb * 4, qb * 4 + 4):
                        def fn(st=st):
                            o_t = ot_half[st]
                            for eb in range(2):
                                ps_f = misc_ps.tile([128, 512], F32, tag="mps",
                                                    name=f"pf{st}{eb}")
                                nc.tensor.matmul(
                                    ps_f[:],
                                    ot_sb[:, S + st * 128: S + st * 128 + 128],
                                    wot_sb[:, E + eb * 512: E + eb * 512 + 512],
                                    start=True, stop=True)
                                nc.vector.tensor_add(
                                    o_t[:, eb * 512:(eb + 1) * 512], ps_f[:],
                                    o_t[:, eb * 512:(eb + 1) * 512])
                            nc.sync.dma_start(
                                out=out_d[st * 128:(st + 1) * 128, :],
                                in_=o_t[:])
                        filler_q.append((NQB, fn))

                qk_tiles = {}

                def queue_qk(sb, pj, dt):
                    # Q/K projection group as two drip closures sharing an
                    # open PSUM accumulation (queue adjacency keeps the
                    # misc_ps rotation from reusing the bank in between)
                    def part_a(sb=sb, pj=pj, dt=dt):
                        psq = misc_ps.tile([128, 512], F32, tag="mps",
                                           name=f"qk{sb}{pj}{dt}")
                        qk_tiles[sb, pj, dt] = psq
                        for ec in range(4):
                            w0 = ec * WQKV + pj * C + dt * 128
                            nc.tensor.matmul(
                                psq[:], wqkv_sb[:, w0: w0 + 128],
                                xt_cols(sb, ec, 0, 512),
                                start=(ec == 0), stop=False,
                                skip_group_check=True)
                    def part_b(sb=sb, pj=pj, dt=dt):
                        psq = qk_tiles.pop((sb, pj, dt))
                        for ec in range(4, NEC):
                            w0 = ec * WQKV + pj * C + dt * 128
                            nc.tensor.matmul(
                                psq[:], wqkv_sb[:, w0: w0 + 128],
                                xt_cols(sb, ec, 0, 512),
                                start=False, stop=(ec == NEC - 1),
                                skip_group_check=True)
                        o_sb = qt_sb if pj == 0 else kt_sb
                        nc.vector.tensor_scalar_add(
                            o_sb[:, dt * S + sb * 512: dt * S + sb * 512 + 512],
                            psq[:], bqk_sb[:, 2 * pj + dt: 2 * pj + dt + 1])
                    filler_q.append((sb, part_a))
                    filler_q.append((sb, part_b))

                if causal:
                    for sb in range(1, NQB):
                        for pj in range(2):
                            for dt in range(2):
                                queue_qk(sb, pj, dt)
                        for st in range(4 * sb, 4 * sb + 4):
                            queue_vproj(st)

                for qb in range(NQB):
                    nk = 4 * (qb + 1) if causal else NST
                    q0 = qb * 512
                    # fillers whose results this q-block reads (V tiles)
                    # must be fully emitted before its first PV
                    drain_due(qb)
                    for hp in range(2):   # head pair (2*hp, 2*hp+1), d-tile hp
                        t = hp
                        ps_os = [None, None]

                        def emit_pv(kt_i, pt, hp=hp, nk=nk):
                            if kt_i == 0:
                                for a in range(2):
                                    ps_os[a] = attn_ps.tile(
                                        [65, 512], F32, tag="po",
                                        name=f"po{qb}{hp}{a}")
                            for a in range(2):
                                h = 2 * hp + a
                                nc.tensor.matmul(
                                    ps_os[a][:],
                                    v_sb[:, kt_i * VW + h * (DH + 1):
                                         kt_i * VW + h * (DH + 1) + DH + 1],
                                    pt[:, a * 512:(a + 1) * 512],
                                    start=(kt_i == 0), stop=(kt_i == nk - 1),
                                    skip_group_check=True)

                        pv_queue = []
                        for kt_i in range(nk):
                            # diagonal tiles: only columns q >= off are
                            # causally valid -- restrict the score matmul
                            # and exp to them, zero the rest, and run the
                            # triangle select on just the 128-wide boundary
                            off = kt_i * 128 - q0
                            diag = causal and off >= 0
                            lo = off if diag else 0
                            ps_s = score_ps.tile([128, 1024], F32, tag="sc",
                                                 name=f"sc{qb}{hp}{kt_i}")
                            pt = pt_pool.tile([128, 1024], BF16, tag="pt",
                                              name=f"pt{qb}{hp}{kt_i}")
                            pt3 = pt[:].rearrange("p (u q) -> p u q", u=2)
                            ps3 = ps_s[:].rearrange("p (u q) -> p u q", u=2)
                            for a in range(2):
                                p0 = a * 64
                                nc.tensor.matmul(
                                    ps_s[:, a * 512 + lo:(a + 1) * 512],
                                    kt_sb[p0:p0 + 64,
                                          t * S + kt_i * 128: t * S + kt_i * 128 + 128],
                                    qt_sb[p0:p0 + 64,
                                          t * S + q0 + lo: t * S + q0 + 512],
                                    start=True, stop=True)
                            nc.scalar.activation(pt3[:, :, lo:],
                                                 ps3[:, :, lo:], ActF.Exp,
                                                 scale=SCALE)
                            if diag:
                                if lo > 0:
                                    nc.gpsimd.memset(pt3[:, :, 0:lo], 0.0)
                                nc.gpsimd.affine_select(
                                    out=pt3[:, :, lo:lo + 128],
                                    in_=pt3[:, :, lo:lo + 128],
                                    compare_op=Alu.is_ge,
                                    fill=0.0, base=0,
                                    pattern=[[0, 2], [1, 128]],
                                    channel_multiplier=-1)
                            if kt_i == 0:
                                # previous hp's norms land here, after this
                                # hp's first scores/exp are in the stream
                                flush_pending()
                            elif kt_i >= 1:
                                # drip filler work between score and PV so
                                # the PE keeps busy while ACT paces exp;
                                # spread a small backlog over the remaining
                                # steps instead of draining it early
                                rem = nk - kt_i
                                if len(filler_q) >= rem:
                                    drip(2)
                                elif 2 * len(filler_q) >= rem or kt_i % 2 == 0:
                                    drip(1)
                            # defer this step's PV by one step: gives the
                            # in-order PE stream slack to clear the norm chain
                            pv_queue.append((kt_i, pt))
                            if len(pv_queue) > 1:
                                emit_pv(*pv_queue.pop(0))
                        while pv_queue:
                            emit_pv(*pv_queue.pop(0))

                        last_qb = qb == NQB - 1

                        def norm(qb=qb, hp=hp, t=t, q0=q0, ps_os=ps_os,
                                 halves=(1 if not last_qb else 2)):
                            # for the last q-block split the norm in halves so
                            # the tail out-projection pipelines per s-tile
                            w = 512 // halves
                            for hh in range(halves):
                                for a in range(2):
                                    h = 2 * hp + a
                                    p0 = a * 64
                                    rs = rec_pool.tile([1, 512], F32R, tag="rs",
                                                       name=f"rs{qb}{h}{hh}")
                                    nc.vector.tensor_copy(
                                        rs[0:1, 0:w],
                                        ps_os[a][64:65, hh * w:(hh + 1) * w])
                                    ps_b = misc_ps.tile([64, 512], F32, tag="mps",
                                                        name=f"pb{qb}{h}{hh}")
                                    nc.tensor.matmul(
                                        ps_b[0:64, 0:w], ones_r[0:1, 0:64],
                                        rs[0:1, 0:w], start=True, stop=True)
                                    bc = bc_pool.tile([64, 512], F32, tag="bc",
                                                      name=f"bc{qb}{h}{hh}")
                                    nc.vector.reciprocal_approx_fast(
                                        bc[0:64, 0:w], ps_b[0:64, 0:w])
                                    nc.vector.tensor_mul(
                                        ot_sb[p0:p0 + 64,
                                              t * S + q0 + hh * w:
                                              t * S + q0 + (hh + 1) * w],
                                        ps_os[a][0:64, hh * w:(hh + 1) * w],
                                        bc[0:64, 0:w])

                        pending.append(norm)
                        if hp == 0:
                            queue_outproj_cc0(qb)
                            if qb == NQB - 1:
                                # keep the HAM clock gate open through the
                                # tail: harmless warm matmuls as fillers
                                for i in range(10):
                                    def warm_fn(i=i):
                                        ps_w = misc_ps.tile(
                                            [128, 128], F32, tag="mps",
                                            name=f"warm{i}")
                                        nc.tensor.matmul(
                                            ps_w[:], dmy_sb[:, 0:128],
                                            dmy_sb[:, 0:128],
                                            start=True, stop=True,
                                            skip_group_check=True)
                                    filler_q.append((NQB, warm_fn))
                        elif qb < NQB - 1:
                            queue_outproj_cc1(qb)
                flush_pending()
                drain_due(NQB)
                # pipelined tail: per s-tile cc1 + store for the last q-block.
                # Adds alternate DVE/gpsimd and stores alternate sync/scalar
                # queues so no single engine serializes the tail.
                qb = NQB - 1
                for st in range(qb * 4, qb * 4 + 4):
                    o_t = ot_half[st]
                    for eb in range(2):
                        ps_f = misc_ps.tile([128, 512], F32, tag="mps",
                                            name=f"pf{st}{eb}")
                        nc.tensor.matmul(
                            ps_f[:],
                            ot_sb[:, S + st * 128: S + st * 128 + 128],
                            wot_sb[:, E + eb * 512: E + eb * 512 + 512],
                            start=True, stop=True)
                        nc.vector.tensor_add(
                            o_t[:, eb * 512:(eb + 1) * 512], ps_f[:],
                            o_t[:, eb * 512:(eb + 1) * 512])
                        dma_eng = nc.sync if eb == 0 else nc.scalar
                        dma_eng.dma_start(
                            out=out_d[st * 128:(st + 1) * 128,
                                      eb * 512:(eb + 1) * 512],
                            in_=o_t[:, eb * 512:(eb + 1) * 512])


def _build(causal):
    nc = bacc.Bacc("TRN2", target_bir_lowering=False, debug=False,
                   num_devices=NCORES)
    with tile.TileContext(nc) as tc:
        _emit(nc, tc, causal)
    nc.compile()
    return nc


def _shard_inputs(QKV, Wq, bq, Wk, bk, Wv, bv, Wo, bo):
    QKV = np.asarray(QKV, dtype=np.float32)
    Wq, Wk, Wv, Wo = (np.asarray(w, dtype=np.float32) for w in (Wq, Wk, Wv, Wo))
    bq, bk, bv, bo = (np.asarray(b_, dtype=np.float32) for b_ in (bq, bk, bv, bo))
    ones = np.ones((1, 128), dtype=np.float32)
    onesv = np.ones((128, NST * HPC), dtype=np.float32)
    in_maps = []
    xt_fold = [
        # [sb, sl, ec, p] -> [p, sb, ec, sl]
        np.ascontiguousarray(
            QKV[b].reshape(NQB, 512, NEC, 128).transpose(3, 0, 2, 1)
            .reshape(128, NQB * NEC * 512)).astype(MM_NP)
        for b in range(B)
    ]
    for core in range(NCORES):
        b, g = divmod(core, TPW)
        cs = slice(g * C, (g + 1) * C)
        bqs, bks = bq[cs], bk[cs]
        bqk = np.stack([bqs[:128], bqs[128:], bks[:128], bks[128:]], axis=1)
        # wqkv fold: [ec, p, (wq|wk|wv) x 256] -> [p, ec, 768]
        w3 = np.concatenate(
            [Wq[cs, :].T.reshape(NEC, 128, C),
             Wk[cs, :].T.reshape(NEC, 128, C),
             Wv[cs, :].T.reshape(NEC, 128, C)], axis=2)
        wqkv = np.ascontiguousarray(
            w3.transpose(1, 0, 2).reshape(128, NEC * WQKV)).astype(MM_NP)
        wot = np.ascontiguousarray(
            Wo[:, cs].T.reshape(2, 128, E).transpose(1, 0, 2)
            .reshape(128, 2 * E)).astype(MM_NP)
        in_maps.append({
            "xt": xt_fold[b],
            "wqkv": wqkv,
            "wot": wot,
            "bqk": np.ascontiguousarray(bqk),
            "bv": bv[cs].reshape(1, C).copy(),
            # host sums the 4 tensor-parallel partials per batch; only one
            # core per group contributes the output bias
            "bo": (bo if g == 0 else np.zeros_like(bo)).reshape(1, E).copy(),
            "ones": ones,
            "onesv": onesv,
        })
    return in_maps


def kernel(QKV, Wq, bq, Wk, bk, Wv, bv, Wo, bo, is_causal):
    causal = bool(int(np.asarray(is_causal)))
    if causal not in _cache:
        _cache[causal] = _build(causal)
    nc = _cache[causal]
    in_maps = _shard_inputs(QKV, Wq, bq, Wk, bk, Wv, bv, Wo, bo)
    res = run_bass_kernel_spmd(nc, in_maps, core_ids=list(range(NCORES)))
    out = np.empty((B, S, E), dtype=np.float32)
    for b in range(B):
        acc = res.results[TPW * b]["out"].astype(np.float32)
        for g in range(1, TPW):
            acc = acc + res.results[TPW * b + g]["out"]
        out[b] = acc
    return out
